# revision 32
# baseline (speedup 1.0000x reference)
"""Trainium2 Bass kernel for nn_Decoder (mask-multiply + Linear(512->16) + overlap-add).

Full-input contract: kernel(mixture_w, est_mask, W) -> [4, 128008] float32.

Sharding: 8 cores = 4 batches x 2 K-halves (8000 frames each).

v3 "folded overlap-add": out[8m+r] = sum_n est[n,m] W[r,n] + sum_n est[n,m-1] W[r+8,n],
so both overlap-add terms accumulate into ONE psum bank per chunk: the W[0:8]
("A") matmuls run against the chunk's est window and the W[8:16] ("B") matmuls
run against the same window shifted one frame left (the chunk is DMA-loaded
with a 1-frame halo). No DVE add, no psB evacuation, no cross-chunk seam.

Per-chunk pipeline (chunk g, all cross-engine consumer stages run >=1 chunk
behind their producer so no semaphore round-trip sits inside one chunk period):
  SP  : wt DMA, then one ~2MB input DMA per chunk (stacked [mw; em] + halo)
  DVE : est = x0 * x1 (f32r out) -- the only per-chunk DVE op
  PE  : 8 accumulating matmuls -> psO[8,W] (complete output block, j-major),
        then 4 transposes of chunk g-1: res -> pst[W/4,32] (k-major)
  ACT : evac psO(g)->res, ct-copy pst(g-1)->ct, output-DMA issue for g-2
Tail (8 samples = W_B^T est[:,last]) is 4 tiny matmuls + direct 32B DMA.
Host adds the 8-sample seam between the two K-halves of each batch.

Every instruction carries at most one semaphore wait (ISA limit)."""

import numpy as np

import concourse.bass as bass
import concourse.mybir as mybir
from concourse.bass_utils import run_bass_kernel_spmd

F32 = mybir.dt.float32
F32R = mybir.dt.float32r

B, N, K, L = 4, 512, 16000, 16
STEP = L // 2              # 8
KLOC = K // 2              # 8000 frames per core
TLOC = STEP * (KLOC - 1) + L   # 64008 local output samples
# frames per chunk (<=500: psum bank; %4==0; count divisible by the xb/ct
# ring depths so bench-loop semaphore sites stay stable; the shrinking tail
# shortens the post-stream pipeline drain)
WIDTHS = [500] * 14 + [400, 200, 160, 120, 80, 40]
assert sum(WIDTHS) == KLOC and all(w % 4 == 0 and w <= 500 for w in WIDTHS)
WMAX = max(WIDTHS)
NX = 4                     # xb ring depth


class _Waiter:
    """Absolute-target waits (single pass) or register-tracked targets with
    constant per-site deltas (inside a bench Fori hardware loop)."""

    def __init__(self, eng):
        self.eng = eng
        self.last = {}
        self.regs = None

    def wait(self, sem, target):
        if self.regs is None:
            self.eng.wait_ge(sem, target)
            self.last[sem.name] = (sem, target)
        else:
            _, prev = self.last[sem.name]
            delta = target - prev
            assert delta >= 0, (sem.name, prev, target)
            self.last[sem.name] = (sem, target)
            reg = self.regs[sem.name]
            if delta:
                self.eng.reg_add(reg, reg, delta)
            self.eng.wait_ge(sem, reg)

    def enter_loop(self):
        self.regs = {}
        for name, (sem, target) in self.last.items():
            reg = self.eng.alloc_register(f"{name}_tgt")
            self.eng.reg_mov(reg, target)
            self.regs[name] = reg


def _build(loops: int | None) -> bass.Bass:
    """loops=None -> graded single-pass kernel (absolute waits only).
    loops>=3 -> bench variant with per-engine Fori steady-state loops."""
    bench = loops is not None
    G = len(WIDTHS)                    # chunks per pass
    starts = np.cumsum([0] + WIDTHS).tolist()   # frame offset per chunk

    nc = bass.Bass()
    x = nc.dram_tensor("x", [2, N, KLOC], F32, kind="ExternalInput")
    # f32r has f32 storage: DMA the [N, L] weight straight into the f32r
    # stationary tile (full-rate PE) with no cast copy.
    wt = nc.dram_tensor("wt", [N, L], F32R, kind="ExternalInput")
    ident = nc.dram_tensor("ident", [8, 8], F32, kind="ExternalInput")
    out = nc.dram_tensor("out", [TLOC], F32, kind="ExternalOutput")

    x_r = x.rearrange("t (ni p) k -> p t ni k", p=128)
    wt_r = wt.rearrange("(ni p) l -> p ni l", p=128)

    from contextlib import ExitStack

    with ExitStack() as stk:
        e = stk.enter_context
        xb = [e(nc.sbuf_tensor(f"xb{i}", [128, 2, 4, WMAX + 1], F32)) for i in range(NX)]
        eb = [e(nc.sbuf_tensor(f"eb{i}", [128, 4, WMAX + 1], F32R)) for i in range(2)]
        wt_sb = e(nc.sbuf_tensor("wt_sb", [128, 4, L], F32R))
        id_sb = e(nc.sbuf_tensor("id_sb", [8, 8], F32))
        res = [e(nc.sbuf_tensor(f"res{i}", [8, WMAX], F32)) for i in range(2)]
        res_tail = e(nc.sbuf_tensor("res_tail", [8, 1], F32))
        # ct ring: the out-DMA completion gate (osem) binds NCT chunks back,
        # so queued output DMAs don't stall the ACT pipeline
        NCT = 4
        ct = [e(nc.sbuf_tensor(f"ct{i}", [WMAX // 4, 32], F32)) for i in range(NCT)]
        psO = [e(nc.psum_tensor(f"psO{i}", [8, WMAX], F32)) for i in range(2)]
        pst = [e(nc.psum_tensor(f"pst{i}", [WMAX // 4, 32], F32)) for i in range(2)]
        psT = e(nc.psum_tensor("psT", [8, 2], F32))
        # Semaphores incremented by MULTIPLE DMAs must be rings: the 16 SDMA
        # engines inc independently, so counts from back-to-back DMAs on one
        # sem interleave and a waiter can fire before the older DMA fully
        # landed. Ring slot g%NX / g%NCT + a transitive gate (msem / the
        # ct-slot wait) orders same-sem DMAs.
        wsem = e(nc.semaphore("wsem"))
        zsem = e(nc.semaphore("zsem"))
        dsem = [e(nc.semaphore(f"dsem{i}")) for i in range(NX)]
        msem = e(nc.semaphore("msem"))
        psem = e(nc.semaphore("psem"))
        tsem = e(nc.semaphore("tsem"))
        esem = e(nc.semaphore("esem"))
        ctsem = e(nc.semaphore("ctsem"))
        osem = [e(nc.semaphore(f"osem{i}")) for i in range(NCT)]
        osem_t = e(nc.semaphore("osem_t"))
        block = e(nc.Block())

        ET = mybir.EngineType

        # Semaphore ledger (g = global chunk index, c = g % G):
        #   mult(g) done  <=> msem = g + 1
        #   MMs(g) done   <=> psem = g + 1 + (tails of completed passes)
        #   evac(g) done  <=> esem = g + 1 + (tail evacs of completed passes)
        #   T(g) done     <=> tsem = g + 1
        #   ct(g) done    <=> ctsem = g + 1
        def psem_after_mm(g):
            return g + 1 + g // G

        def psem_after_tail(g):
            return g + 2 + g // G

        def esem_after_evac(g):
            return g + 1 + g // G

        def esem_after_tail(g):
            return g + 2 + g // G

        def loop_or_unroll(W, engine_type, chunk_fn, lo=0, hi=None):
            """Emit chunk_fn(lo..hi-1) unrolled (single pass), or peel two
            passes then Fori over the rest (bench)."""
            if not bench:
                for g in range(lo, hi if hi is not None else G):
                    chunk_fn(g)
                return
            for g in range(2 * G):
                chunk_fn(g)
            W.enter_loop()
            with nc.Fori(2, loops, engines=[engine_type]):
                for cc in range(G):
                    chunk_fn(2 * G + cc)

        @block.sync
        def _(sync):
            W = _Waiter(sync)

            def chunk(g):
                c = g % G
                w = WIDTHS[c]
                f = starts[c]
                if g >= NX:
                    W.wait(msem, g - NX + 1)   # mult(g-NX) done reading xb[g%NX]
                if c == 0:
                    src = x_r[:, :, :, 0:w]
                    dst = xb[g % NX][:, :, :, 1 : w + 1]
                else:
                    src = x_r[:, :, :, f - 1 : f + w]
                    dst = xb[g % NX][:, :, :, 0 : w + 1]
                sync.dma_start(dst, src).then_inc(dsem[g % NX], 16)

            loop_or_unroll(W, ET.SP, chunk)
            if not bench:
                sync.wait_ge(esem, G + 1)   # tail evac done
                sync.dma_start(
                    out[STEP * KLOC : TLOC].rearrange("(p x) -> p x", x=1),
                    res_tail[:],
                ).then_inc(osem_t, 16)
                g_ep = G - 2
                w = WIDTHS[g_ep]
                f = starts[g_ep]
                dst = out[8 * f : 8 * f + 8 * w].rearrange(
                    "(p t j) -> p t j", p=w // 4, t=4
                )
                sync.wait_ge(ctsem, g_ep + 1)
                sync.dma_start(
                    dst,
                    ct[g_ep % NCT][0 : w // 4, :].rearrange("p (t j) -> p t j", t=4),
                ).then_inc(osem[g_ep % NCT], 16)

        @block.vector
        def _(vector):
            W = _Waiter(vector)
            # zero the chunk-0 halo column (frame -1); the full-width mult
            # propagates it into eb as est[:, -1] = 0. DVE memset completion
            # is not ordered with later TensorTensor reads: sync explicitly.
            nc.vector.memset(xb[0][:, :, :, 0:1], 0.0).then_inc(zsem, 1)
            vector.wait_ge(zsem, 1)

            def chunk(g):
                c = g % G
                w = WIDTHS[c]
                b = g % 2
                W.wait(dsem[g % NX], 16 * (g // NX + 1))
                if g >= 2:
                    # eb[b] free: last read by MMs(g-2) (+ tail MMs if g-2
                    # ended a pass)
                    if (g - 2) % G == G - 1:
                        W.wait(psem, psem_after_tail(g - 2))
                    else:
                        W.wait(psem, psem_after_mm(g - 2))
                nc.vector.tensor_mul(
                    out=eb[b][:, :, 0 : w + 1],
                    in0=xb[g % NX][:, 0, :, 0 : w + 1],
                    in1=xb[g % NX][:, 1, :, 0 : w + 1],
                ).then_inc(msem, 1)

            loop_or_unroll(W, ET.DVE, chunk)

        @block.tensor
        def _(tensor):
            W = _Waiter(tensor)
            tensor.wait_ge(wsem, 32)   # wt_sb + id_sb loaded

            def transposes(g):
                b = g % 2
                w = WIDTHS[g % G]
                W.wait(esem, esem_after_evac(g))   # res[b] written by evac(g)
                if g >= 2:
                    W.wait(ctsem, g - 1)           # pst[b] free (ct(g-2) done)
                for t in range(4):
                    tr = nc.tensor.transpose(
                        pst[b][0 : w // 4, 8 * t : 8 * t + 8],
                        res[b][:, t:w:4],
                        id_sb[:],
                    )
                    if t == 3:
                        tr.then_inc(tsem, 1)

            def chunk(g):
                c = g % G
                w = WIDTHS[c]
                b = g % 2
                W.wait(msem, g + 1)                    # mult(g) done
                if g >= 2:
                    W.wait(esem, esem_after_evac(g - 2))  # psO[b] free
                for ni in range(4):
                    nc.tensor.matmul(
                        psO[b][:, 0:w], wt_sb[:, ni, 0:STEP], eb[b][:, ni, 1 : w + 1],
                        start=(ni == 0), stop=False,
                    )
                for ni in range(4):
                    mm = nc.tensor.matmul(
                        psO[b][:, 0:w], wt_sb[:, ni, STEP:L], eb[b][:, ni, 0:w],
                        start=False, stop=(ni == 3),
                    )
                    if ni == 3:
                        mm.then_inc(psem, 1)
                if c == G - 1:
                    # tail: W_B^T est[:, last frame] -> psT[:, 1] (width 2:
                    # fp32r matmuls need an even moving width; col 0 is junk)
                    for ni in range(4):
                        mm = nc.tensor.matmul(
                            psT[:], wt_sb[:, ni, STEP:L], eb[b][:, ni, w - 1 : w + 1],
                            start=(ni == 0), stop=(ni == 3),
                        )
                        if ni == 3:
                            mm.then_inc(psem, 1)
                if g >= 1:
                    transposes(g - 1)

            loop_or_unroll(W, ET.PE, chunk)
            if not bench:
                transposes(G - 1)

        @block.scalar
        def _(scalar):
            W = _Waiter(scalar)
            scalar.dma_start(wt_sb[:], wt_r).then_inc(wsem, 16)
            scalar.dma_start(id_sb[:], ident[:]).then_inc(wsem, 16)

            def ct_copy(g):
                b = g % 2
                w = WIDTHS[g % G]
                W.wait(tsem, g + 1)                # T(g) done
                if g >= NCT:
                    W.wait(osem[g % NCT], 16 * (g // NCT))  # ct slot free
                nc.scalar.copy(
                    out=ct[g % NCT][0 : w // 4, :], in_=pst[b][0 : w // 4, :]
                ).then_inc(ctsem, 1)

            def out_dma(g):
                c = g % G
                w = WIDTHS[c]
                f = starts[c]
                dst = out[8 * f : 8 * f + 8 * w].rearrange(
                    "(p t j) -> p t j", p=w // 4, t=4
                )
                # the DMA trigger is async wrt the ACT pipe: gate on ctsem
                W.wait(ctsem, g + 1)
                scalar.dma_start(
                    dst, ct[g % NCT][0 : w // 4, :].rearrange("p (t j) -> p t j", t=4)
                ).then_inc(osem[g % NCT], 16)

            def chunk(g):
                c = g % G
                w = WIDTHS[c]
                b = g % 2
                W.wait(psem, psem_after_mm(g))
                if g >= 2:
                    W.wait(tsem, g - 1)            # res[b] free (T(g-2) done)
                nc.scalar.copy(out=res[b][:, 0:w], in_=psO[b][:, 0:w]).then_inc(esem, 1)
                if c == G - 1:   # tail evac (its completion wait overlaps below)
                    W.wait(psem, psem_after_tail(g))
                    W.wait(osem_t, 16 * (g // G))  # res_tail free (prev pass)
                    nc.scalar.copy(out=res_tail[:], in_=psT[:, 1:2]).then_inc(esem, 1)
                if g >= 1:
                    ct_copy(g - 1)
                if g >= 2:
                    out_dma(g - 2)
                if bench and c == G - 1:   # 32B tail output DMA (in-loop)
                    W.wait(esem, esem_after_tail(g))
                    scalar.dma_start(
                        out[STEP * KLOC : TLOC].rearrange("(p x) -> p x", x=1),
                        res_tail[:],
                    ).then_inc(osem_t, 16)

            loop_or_unroll(W, ET.Activation, chunk)
            if not bench:
                # final DMAs split across both rings: tail + G-2 issue from
                # the idle sync engine in parallel with ct(G-1) -> dma(G-1)
                ct_copy(G - 1)
                out_dma(G - 1)

    return nc


def build_nc():
    return _build(None)


def build_bench_nc(loops):
    return _build(loops)


def audit_waits(nc, max_show=12):
    """Count on_wait entries per instruction; the TPB ISA allows ONE."""
    import json

    d = json.loads(nc.to_json_bytes())
    bad = []

    def walk(blocks):
        for bb in blocks:
            for i in bb.get("instructions", []):
                si = i.get("sync_info") or {}
                w = si.get("on_wait") or []
                if len(w) > 1:
                    bad.append(
                        (
                            i["name"],
                            i.get("opcode"),
                            len(w),
                            [s_.get("ant_name") for s_ in w],
                        )
                    )
            walk(bb.get("blocks", []))

    walk(d["functions"][0]["blocks"])
    return bad[:max_show], len(bad)


_NC_CACHE = {}


def _get_nc():
    if "v3" not in _NC_CACHE:
        _NC_CACHE["v3"] = build_nc()
    return _NC_CACHE["v3"]


def make_in_maps(mixture_w, est_mask, W):
    mixture_w = np.asarray(mixture_w, dtype=np.float32)
    est_mask = np.asarray(est_mask, dtype=np.float32)
    W = np.asarray(W, dtype=np.float32)
    wt = np.ascontiguousarray(W.T)                      # [N, L]
    ident = np.eye(8, dtype=np.float32)
    in_maps = []
    for c in range(8):
        b, h = c // 2, c % 2
        xx = np.stack(
            [
                mixture_w[b, :, h * KLOC : (h + 1) * KLOC],
                est_mask[b, :, h * KLOC : (h + 1) * KLOC],
            ]
        )
        in_maps.append({"x": np.ascontiguousarray(xx), "wt": wt, "ident": ident})
    return in_maps


def assemble(results):
    T = STEP * (K - 1) + L
    out = np.zeros((B, T), dtype=np.float32)
    for c in range(8):
        b, h = c // 2, c % 2
        out[b, h * STEP * KLOC : h * STEP * KLOC + TLOC] += results[c]["out"]
    return out


def run(mixture_w, est_mask, W, trace=False, **spmd_kwargs):
    """Shard, run on 8 cores, gather. Returns (out, BassKernelResults)."""
    in_maps = make_in_maps(mixture_w, est_mask, W)
    nc = _get_nc()
    kr = run_bass_kernel_spmd(
        nc, in_maps, core_ids=list(range(8)), trace=trace, **spmd_kwargs
    )
    return assemble(kr.results), kr


def kernel(mixture_w, est_mask, W):
    out, _ = run(mixture_w, est_mask, W)
    return out
